# revision 5
# baseline (speedup 1.0000x reference)
"""Trainium2 Bass kernel for batched multi-head attention (no scale).

Problem: q,k,v [B=4, H=16, S=2048, D=128] fp32;
    out = softmax(q @ k^T) @ v   (no 1/sqrt(D) scaling)

Sharding: B*H = 64 heads, 8 heads per core across 8 NeuronCores.

v2 design (vs. the P-export baseline at 279.8us):
  The baseline was a three-way tie: ScalarE exp 257.7us, PE matmul 258.1us,
  DMA 88.3MB ~236us (67MB of it the full-P export for the host-side softmax
  denominator).  v2 removes the P export entirely and rebalances:

  * Denominator on device: the 16 P blocks of a q-supertile are pairwise
    tree-folded on the (idle) Vector engine in bf16 (validated: adds <1e-4
    to rel err), and the folded [128, 1024] tile is exported (4MB/core);
    the host does the final 128-partition fold + divide.  DMA drops to
    ~25MB/core.
  * ScalarE: exp ACTs are pair-fused to FD=2048 (amortizes the ~185ns fixed
    ACT overhead) by reading TWO PSUM logit slots with one multi-dim AP --
    slots rotate over a persistent [128, 3, 1024] fp32 PSUM tile (6 banks);
    wrapped pairs (slot2,slot0) use a negative-stride AP.  ScalarE drops
    ~258 -> ~242us and stays the (unavoidable: exp is 1 elem/cycle/lane,
    ScalarE-only) bottleneck.
  * PE: q-supertiles of 1024 make every stationary operand (K-block for QK,
    V-block for AV) serve 2 N=512 matmuls, halving LDWEIGHTS pressure.
  * Warmup: dummy matmuls + a dummy exp during the initial DMA wait warm the
    PE HAM clock-gate (1.2->2.4GHz) and preload the ACT exp table.

dtype choices: Q,K bf16, V fp16, P bf16 (rel err ~8.3e-3, gate 2e-2).
Host pre-transposes Q,K to [D,S] and pre-swizzles V to [128, NKB, D] fp16;
post-applies out = (out^T / l)^T with l from the exported folded P.
"""

import os

import ml_dtypes
import numpy as np

import concourse.bass as bass
import concourse.tile as tile
from concourse import bacc, mybir
from concourse.bass_utils import run_bass_kernel_spmd

B, H, S, D = 4, 16, 2048, 128
N_CORES = 8
HPC = (B * H) // N_CORES  # heads per core
QT = 1024                 # q-supertile width
NQT = S // QT             # 2 supertiles per head
KB = 128                  # kk block (contraction of one matmul)
NKB = S // KB             # 16 kk blocks
NPAIR = NKB // 2          # 8 exp-pairs per supertile
EXP_BIAS = -64.0
F32 = mybir.dt.float32
BF16 = mybir.dt.bfloat16
FP16 = mybir.dt.float16

_NC_CACHE = None


def _build_nc():
    nc = bacc.Bacc("TRN2", target_bir_lowering=False, debug=False)

    qT_d = nc.dram_tensor("qT", [HPC, D, S], BF16, kind="ExternalInput")
    kT_d = nc.dram_tensor("kT", [HPC, D, S], BF16, kind="ExternalInput")
    v_d = nc.dram_tensor("v", [HPC, 128, NKB, D], FP16, kind="ExternalInput")
    oT_d = nc.dram_tensor("outT", [HPC, D, S], F32, kind="ExternalOutput")
    accf_d = nc.dram_tensor(
        "acc_fold", [HPC, NQT, 128, QT], BF16, kind="ExternalOutput"
    )

    with tile.TileContext(nc) as tc:
        with (
            tc.tile_pool(name="io", bufs=2) as io,
            tc.tile_pool(name="pexp", bufs=4) as pexp,
            tc.tile_pool(name="fold", bufs=2) as foldp,
            tc.tile_pool(name="osb", bufs=2) as osbp,
            tc.tile_pool(name="small", bufs=1) as small,
            tc.tile_pool(name="ps", bufs=1, space="PSUM") as ps,
        ):
            bias_sb = small.tile([128, 1], F32)
            nc.vector.memset(bias_sb[:], EXP_BIAS)

            # --- warmup: PE HAM + ACT exp table, during the initial DMA wait
            wu_w = small.tile([128, 128], BF16, name="wu_w")
            wu_r = small.tile([128, 512], BF16, name="wu_r")
            wu_o = small.tile([128, 128], BF16, name="wu_o")
            nc.vector.memset(wu_w[:], 0.0)
            nc.vector.memset(wu_r[:], 0.0)

            # persistent PSUM: 3 logit slots (6 banks) + AV accumulator (2)
            st3 = ps.tile([128, 3, QT], F32, tag="st3", bufs=1, name="st3")

            # ACT table preload (no data deps)
            nc.scalar.activation(
                wu_o[:],
                wu_w[:],
                mybir.ActivationFunctionType.Exp,
                bias=bias_sb[:, :],
                scale=1.0,
            )
            # PE warmup matmuls (~4us of PE activity from t=0)
            for i in range(18):
                nc.tensor.matmul(
                    st3[:, i % 3, 0:512], wu_w[:], wu_r[:], start=True, stop=True
                )

            # --- per-head input DMAs.  head 0 arrives in fine-grained chunks
            # so the first QK matmuls can start ~5us earlier.
            def load_head(hd):
                qT_sb = io.tile([128, S], BF16, tag="qT", name="qT")
                kT_sb = io.tile([128, S], BF16, tag="kT", name="kT")
                v_sb = io.tile([128, NKB, D], FP16, tag="v", name="v")
                dma = nc.default_dma_engine
                if hd == 0:
                    # critical-path-first: K blocks 0-1, first q half...
                    kc, qc, vc = 256, 512, 2
                    for c in range(S // kc):
                        sl = slice(c * kc, (c + 1) * kc)
                        dma.dma_start(out=kT_sb[:, sl], in_=kT_d[hd, :, sl])
                        if c * qc < S:
                            slq = slice(c * qc, (c + 1) * qc)
                            dma.dma_start(out=qT_sb[:, slq], in_=qT_d[hd, :, slq])
                        lo = c * vc
                        dma.dma_start(
                            out=v_sb[:, lo:lo + vc, :], in_=v_d[hd, :, lo:lo + vc, :]
                        )
                else:
                    dma.dma_start(out=qT_sb[:], in_=qT_d[hd])
                    dma.dma_start(out=kT_sb[:], in_=kT_d[hd])
                    dma.dma_start(out=v_sb[:], in_=v_d[hd])
                return qT_sb, kT_sb, v_sb

            heads = {0: load_head(0)}

            # --- software pipeline over global pair-tasks
            # task t: QK(pair t) -> ACT(t-1) -> AV(t-2) + fold(t-2)
            T = HPC * NQT * NPAIR  # 128

            def task(t):
                hd, r = divmod(t, NQT * NPAIR)
                sti, p = divmod(r, NPAIR)
                return hd, sti, p

            p_hist = {}     # pair tasks -> P tile [128, 2, 1024]
            strip = {}      # supertile-local strip sums
            acc_cur = {}    # (hd, sti) -> AV accumulator psum tile

            def emit_qk_block(t, j):
                hd, sti, p = task(t)
                qT_sb, kT_sb, _ = heads[hd]
                b = 2 * p + j
                gb = 2 * t + j
                slot = gb % 3
                kblk = kT_sb[:, b * KB:(b + 1) * KB]
                for h in range(2):
                    nc.tensor.matmul(
                        st3[:, slot, h * 512:(h + 1) * 512],
                        kblk,
                        qT_sb[:, sti * QT + h * 512: sti * QT + (h + 1) * 512],
                        start=True,
                        stop=True,
                    )

            def emit_act(t):
                hd, sti, p = task(t)
                gb = 2 * t
                s0 = gb % 3
                if s0 <= 1:
                    in_ap = st3[:, s0:s0 + 2, :]
                else:
                    in_ap = st3[:, 2::-2, :]  # slot2 then slot0 (wrap)
                p_sb = pexp.tile([128, 2, QT], BF16, tag="p", name="p")
                nc.scalar.activation(
                    p_sb[:],
                    in_ap,
                    mybir.ActivationFunctionType.Exp,
                    bias=bias_sb[:, :],
                    scale=1.0,
                )
                p_hist[t] = p_sb

            def emit_av(t):
                hd, sti, p = task(t)
                _, _, v_sb = heads[hd]
                p_sb = p_hist[t]
                if p == 0:
                    acc_cur[(hd, sti)] = ps.tile([128, QT], F32, tag="acc", bufs=1, name="acc")
                acc = acc_cur[(hd, sti)]
                for j in range(2):
                    b = 2 * p + j
                    vblk = v_sb[:, b, :]
                    for h in range(2):
                        nc.tensor.matmul(
                            acc[:, h * 512:(h + 1) * 512],
                            vblk,
                            p_sb[:, j, h * 512:(h + 1) * 512],
                            start=(b == 0),
                            stop=(b == NKB - 1),
                        )

            def emit_fold(t):
                # strip-fold P(t): [128,2,1024] -> [128,1024]; then tree
                hd, sti, p = task(t)
                p_sb = p_hist.pop(t)
                s = foldp.tile([128, QT], BF16, tag=f"f{p % 2}", name="fs")
                nc.vector.tensor_add(s[:], p_sb[:, 0, :], p_sb[:, 1, :])
                strip[p] = s
                if p % 2 == 1:
                    u = foldp.tile([128, QT], BF16, tag=f"t{(p // 2) % 2}", name="ft")
                    nc.vector.tensor_add(u[:], strip.pop(p - 1)[:], strip.pop(p)[:])
                    strip[8 + p // 2] = u
                if p % 4 == 3:
                    w = foldp.tile([128, QT], BF16, tag=f"u{(p // 4) % 2}", name="fu")
                    nc.vector.tensor_add(
                        w[:], strip.pop(8 + p // 2 - 1)[:], strip.pop(8 + p // 2)[:]
                    )
                    strip[12 + p // 4] = w
                if p == NPAIR - 1:
                    af = foldp.tile([128, QT], BF16, tag="af", name="af")
                    nc.vector.tensor_add(af[:], strip.pop(12)[:], strip.pop(13)[:])
                    nc.default_dma_engine.dma_start(
                        out=accf_d[hd, sti], in_=af[:]
                    )
                    # evacuate the AV accumulator and export out^T
                    acc = acc_cur.pop((hd, sti))
                    out_sb = osbp.tile([128, QT], F32, tag="osb", name="osb")
                    nc.vector.tensor_copy(out_sb[:], acc[:])
                    nc.default_dma_engine.dma_start(
                        out=oT_d[hd, :, sti * QT:(sti + 1) * QT], in_=out_sb[:]
                    )

            for t in range(T + 2):
                if t < T:
                    hd, sti, p = task(t)
                    # prefetch next head one full head ahead
                    if sti == 0 and p == 0 and hd + 1 < HPC:
                        heads[hd + 1] = load_head(hd + 1)
                if t >= 1 and t - 1 < T:
                    emit_act(t - 1)
                if t < T:
                    emit_qk_block(t, 0)
                if t >= 2:
                    emit_av(t - 2)
                if t < T:
                    emit_qk_block(t, 1)
                if t >= 2:
                    emit_fold(t - 2)

    nc.finalize()
    return nc


def _get_nc():
    global _NC_CACHE
    if _NC_CACHE is None:
        _NC_CACHE = _build_nc()
    return _NC_CACHE


def kernel(q, k, v):
    q = np.asarray(q, dtype=np.float32).reshape(B * H, S, D)
    k = np.asarray(k, dtype=np.float32).reshape(B * H, S, D)
    v = np.asarray(v, dtype=np.float32).reshape(B * H, S, D)

    in_maps = []
    for c in range(N_CORES):
        sl = slice(c * HPC, (c + 1) * HPC)
        vh = v[sl].reshape(HPC, NKB, 128, D).transpose(0, 2, 1, 3)
        in_maps.append(
            {
                "qT": np.ascontiguousarray(q[sl].transpose(0, 2, 1)).astype(
                    ml_dtypes.bfloat16
                ),
                "kT": np.ascontiguousarray(k[sl].transpose(0, 2, 1)).astype(
                    ml_dtypes.bfloat16
                ),
                "v": np.ascontiguousarray(vh).astype(np.float16),
            }
        )

    nc = _get_nc()
    trace = bool(int(os.environ.get("KERNEL_TRACE", "0")))
    res = run_bass_kernel_spmd(
        nc, in_maps, core_ids=list(range(N_CORES)), trace=trace
    )
    if trace:
        print(f"HW exec time: {res.exec_time_ns} ns")
        if res.instructions_and_trace:
            print(f"Trace: {res.instructions_and_trace[1]}")

    out = np.empty((B * H, S, D), dtype=np.float32)
    for c in range(N_CORES):
        oT = res.results[c]["outT"]  # [HPC, D, S]
        accf = np.asarray(res.results[c]["acc_fold"]).astype(np.float32)
        # final 128-partition fold of the device-side pairwise-folded P
        l = accf.sum(axis=2).reshape(HPC, S)  # [HPC, NQT, QT] -> [HPC, S]
        out[c * HPC:(c + 1) * HPC] = oT.transpose(0, 2, 1) / l[:, :, None]
    return out.reshape(B, H, S, D)
